# revision 21
# baseline (speedup 1.0000x reference)
"""BIDAF attention-flow kernel for Trainium2 (Bass/Tile), 8-core data-parallel.

Reference computation (per batch b):
    S[t,j]  = H[t]·w_h + U[j]·w_u + sum_d H[t,d]*U[j,d]*w_hu[d]
    A       = softmax_j(S);          C2Q = A @ U
    b_att   = softmax_t(max_j S);    Q2C = b_att @ H   (broadcast over t)
    G       = [H, C2Q, H*C2Q, H*Q2C]        # [T, 4D]

v2 strategy (per core, 8 batches), designed around the HBM roofline
(~5.9 MB/batch -> ~132 us/core at 358 GB/s) with all transposes removed:
  * H is supplied twice from the host: t-major fp32 (augmented with a ones
    column for the Q2C normalizer) and d-major bf16 (for the similarity
    matmuls).  U likewise (j-major bf16 + d-major bf16).
  * w_h is folded into the similarity weights: Uw'[d,j] = U[j,d]*w_hu[d] +
    w_h[d], so S' = shu + sh comes out of one matmul; su is a per-partition
    ACT bias for the softmax_j exp, and is folded into the t-major S via a
    K=1 ones-matmul for the max_j path.
  * S is computed in BOTH orientations on the PE (cheap with bf16 weights /
    fast-weight-load) instead of PE-transposing P: [j,t] for softmax_j ->
    C2Q, and [t,j] so max_j is a free-dim DVE reduce.
  * ones-columns give l[t]=sum_j P and Wsum=sum_t wq inside the C2Q/Q2C
    matmuls; normalizers fold into the mandatory PSUM->SBUF copies.
  * elementwise G blocks are split across DVE and GpSimd.
"""

import os
import sys

sys.path.insert(0, "/opt/trn_rl_repo")

import numpy as np

import concourse.bass as bass
import concourse.mybir as mybir
from concourse import tile

B, T, J, D = 64, 1024, 128, 256
NCORES = 8
BPC = B // NCORES  # batches per core
P = 128
NT = T // P  # 8 t-chunks per batch
DA = 258  # augmented feature dim: [x | 1 | pad(1.0)]
F32 = mybir.dt.float32
F32R = mybir.dt.float32r
BF16 = mybir.dt.bfloat16
AF = mybir.ActivationFunctionType
ALU = mybir.AluOpType
AX = mybir.AxisListType

PHASE = int(os.environ.get("KPHASE", "10"))
KDBG = int(os.environ.get("KDBG", "0"))


def build_kernel(nc, bpc):
    # per-partition-contiguous combined buffers: one load each for H and U
    HC = nc.declare_dram_parameter("HC", [bpc, P, NT * DA + 2 * T], BF16, isOutput=False)
    UC = nc.declare_dram_parameter("UC", [bpc, P, DA + 2 * P], BF16, isOutput=False)
    ones_in = nc.declare_dram_parameter("ones1", [1, P], BF16, isOutput=False)
    wu_in = nc.declare_dram_parameter("wu_col", [P, 2, 1], BF16, isOutput=False)
    ww_in = nc.declare_dram_parameter("whu_wh", [P, 2, 2], F32, isOutput=False)
    G = nc.declare_dram_parameter("G", [bpc, T, 4 * D], F32, isOutput=True)
    if KDBG:
        Dsu = nc.declare_dram_parameter("Dsu", [bpc, 1, P], F32, isOutput=True)
        Dmx = nc.declare_dram_parameter("Dmx", [bpc, P, NT], F32, isOutput=True)
        Dwq = nc.declare_dram_parameter("Dwq", [bpc, P, NT], F32, isOutput=True)
        Dqc = nc.declare_dram_parameter("Dqc", [bpc, 1, 257], F32, isOutput=True)
        Dqb = nc.declare_dram_parameter("Dqb", [bpc, P, D], F32, isOutput=True)

    with tile.TileContext(nc) as tc:
        with (
            tc.tile_pool(name="const", bufs=1) as const_pool,
            tc.tile_pool(name="h", bufs=3) as h_pool,
            tc.tile_pool(name="ht", bufs=3) as ht_pool,
            tc.tile_pool(name="p", bufs=3) as p_pool,
            tc.tile_pool(name="g", bufs=3) as g_pool,
            tc.tile_pool(name="u", bufs=2) as u_pool,
            tc.tile_pool(name="sm", bufs=2) as sm_pool,
            tc.tile_pool(name="stps", bufs=1, space="PSUM") as st_ps,
            tc.tile_pool(name="st2ps", bufs=1, space="PSUM") as st2_ps,
            tc.tile_pool(name="cqps", bufs=2, space="PSUM") as cq_ps,
            tc.tile_pool(name="smps", bufs=2, space="PSUM") as sm_ps,
        ):
            # ---- constants ----
            ones1 = const_pool.tile([1, P], BF16)
            nc.scalar.dma_start(ones1[:], ones_in[:])
            wu_col = const_pool.tile([P, 2, 1], BF16)
            nc.scalar.dma_start(wu_col[:], wu_in[:])
            whu_wh = const_pool.tile([P, 2, 2], F32)
            nc.scalar.dma_start(whu_wh[:], ww_in[:])

            for b in range(bpc):
                # ---- load inputs ----
                # one combined load: t-major Hb (Q2C rhs, G3/G4 muls, G0
                # cast-store) then d-major HT (similarity matmuls) as views
                Hc = h_pool.tile([P, NT * DA + 2 * T], BF16)
                nc.scalar.dma_start(Hc[:], HC[b])
                Hb = Hc[:, 0 : NT * DA].rearrange("p (c d) -> p c d", d=DA)
                HT = Hc[:, NT * DA :].rearrange("p (k t) -> p k t", t=T)
                Uc = u_pool.tile([P, DA + 2 * P], BF16)
                nc.scalar.dma_start(Uc[:], UC[b])
                Uo = Uc[:, 0:DA]
                UT = Uc[:, DA:].rearrange("p (k j) -> p k j", j=P)

                # G block 0 = H via SWDGE cast-DMA (earliest store: feeds
                # the DMA stream while compute runs)
                Gb = G[b].rearrange("(c p) (g d) -> p c g d", p=P, d=D)
                nc.gpsimd.dma_start(Gb[:, :, 0, :], Hb[:, :, 0:D])  # SWDGE lane

                if PHASE < 2:
                    continue
                # ---- U-side prep ----
                # Uw'[d,j] = U[j,d]*w_hu[d] + w_h[d]  (folds sh into S)
                UwT = u_pool.tile([P, 2, P], BF16)
                for kc in range(2):
                    nc.vector.tensor_scalar(
                        UwT[:, kc, :],
                        UT[:, kc, :],
                        whu_wh[:, kc, 0:1],
                        whu_wh[:, kc, 1:2],
                        op0=ALU.mult,
                        op1=ALU.add,
                    )
                # su as a row [1, j] (for the t-major S fold)...
                sur_ps = sm_ps.tile([1, P], F32, tag="sm")
                for kc in range(2):
                    nc.tensor.matmul(
                        sur_ps[:],
                        wu_col[:, kc, :],
                        UT[:, kc, :],
                        start=(kc == 0),
                        stop=(kc == 1),
                    )
                su_row = sm_pool.tile([1, P], BF16)
                nc.scalar.copy(su_row[:], sur_ps[:])
                if KDBG:
                    su_f = sm_pool.tile([1, P], F32)
                    nc.vector.tensor_copy(su_f[:], su_row[:])
                    nc.sync.dma_start(Dsu[b], su_f[:])
                # ...and as a column [j, 1] (ACT bias for the softmax_j exp)
                suc_ps = sm_ps.tile([P, 1], F32, tag="sm")
                for kc in range(2):
                    nc.tensor.matmul(
                        suc_ps[:],
                        UT[:, kc, :],
                        wu_col[:, kc, :],
                        start=(kc == 0),
                        stop=(kc == 1),
                    )
                su_col = sm_pool.tile([P, 1], F32)
                nc.scalar.copy(su_col[:], suc_ps[:])

                if PHASE < 5:
                    continue
                # ---- S2 [t, j] per t-chunk; su folded via K=1 ones-matmul ----
                # NOTE: start=True clears has_written for the WHOLE PSUM bank,
                # so each chunk's accumulation group must complete before the
                # next chunk's start — emit per-chunk [su-fold, kc0, kc1].
                st2 = st2_ps.tile([P, NT, P], F32, tag="st2")
                for c in range(NT):
                    nc.tensor.matmul(
                        st2[:, c, :],
                        ones1[:],
                        su_row[:],
                        start=True,
                        stop=False,
                    )
                    for kc in range(2):
                        nc.tensor.matmul(
                            st2[:, c, :],
                            HT[:, kc, c * P : (c + 1) * P],
                            UwT[:, kc, :],
                            start=False,
                            stop=(kc == 1),
                        )
                # ---- wq = exp(max_j S2) ----
                mx = sm_pool.tile([P, NT], F32)
                nc.vector.reduce_max(mx[:].unsqueeze(2), st2[:], axis=AX.X)
                wq = sm_pool.tile([P, NT], BF16)
                nc.scalar.activation(wq[:], mx[:], AF.Exp)
                if KDBG:
                    nc.sync.dma_start(Dmx[b], mx[:])
                    nc.sync.dma_start(Dwq[b], wq[:])

                if PHASE < 5:
                    continue
                # ---- S1 [j, t] = Uw'.T @ H.T  (+ sh via the fold) ----
                st = st_ps.tile([P, T], F32, tag="st")
                for kc in range(2):
                    for th in range(2):
                        nc.tensor.matmul(
                            st[:, th * 512 : (th + 1) * 512],
                            UwT[:, kc, :],
                            HT[:, kc, th * 512 : (th + 1) * 512],
                            start=(kc == 0),
                            stop=(kc == 1),
                        )

                if PHASE < 5:
                    continue
                # ---- P = exp(S1 + su[j]) ----
                Pt = p_pool.tile([P, T], BF16)
                nc.scalar.activation(Pt[:], st[:], AF.Exp, bias=su_col[:], scale=1.0)

                if PHASE < 6:
                    continue
                # ---- Q2C (serial chain, front-loaded): wq @ H / Wsum ----
                q2cu_ps = sm_ps.tile([1, DA], F32, tag="sm")
                for c in range(NT):
                    nc.tensor.matmul(
                        q2cu_ps[:],
                        wq[:, c : c + 1],
                        Hb[:, c, :],
                        start=(c == 0),
                        stop=(c == NT - 1),
                    )
                q2cu = sm_pool.tile([1, 257], F32)
                nc.scalar.copy(q2cu[:], q2cu_ps[:, 0:257])
                if KDBG:
                    nc.sync.dma_start(Dqc[b], q2cu[:])
                rin = sm_pool.tile([1, 1], F32)
                nc.vector.reciprocal(rin[:], q2cu[:, 256:257])
                q2cn = sm_pool.tile([1, D], BF16)
                nc.scalar.activation(q2cn[:], q2cu[:, 0:256], AF.Copy, scale=rin[:])
                # broadcast Q2C across partitions with a K=1 ones-matmul
                qb_ps = sm_ps.tile([P, D], F32, tag="sm")
                nc.tensor.matmul(qb_ps[:], ones1[:], q2cn[:], start=True, stop=True)
                q2cb = sm_pool.tile([P, D], F32)
                nc.scalar.copy(q2cb[:], qb_ps[:])
                if KDBG:
                    nc.sync.dma_start(Dqb[b], q2cb[:])

                if PHASE < 7:
                    continue
                # ---- G34 = [H*C2Q, H*Q2C] combined tile (2KB store chunks);
                # G4 parts first (q2cb is ready), G3 parts after C2Q ----
                G34 = g_pool.tile([P, NT, 2, D], F32)
                nc.gpsimd.tensor_mul(
                    G34[:, 0:4, 1, :],
                    Hb[:, 0:4, 0:D],
                    q2cb[:].unsqueeze(1).broadcast_to((P, 4, D)),
                )
                nc.vector.tensor_mul(
                    G34[:, 4:8, 1, :],
                    Hb[:, 4:8, 0:D],
                    q2cb[:].unsqueeze(1).broadcast_to((P, 4, D)),
                )

                if PHASE < 8:
                    continue
                # ---- C2Q chunks ----
                C2Q = g_pool.tile([P, NT, D], F32)
                linv = sm_pool.tile([P, NT], F32)
                for c in range(NT):
                    cq = cq_ps.tile([P, DA], F32, tag="cq")
                    nc.tensor.matmul(
                        cq[:],
                        Pt[:, c * P : (c + 1) * P],
                        Uo,
                        start=True,
                        stop=True,
                    )
                    nc.vector.reciprocal(linv[:, c : c + 1], cq[:, 256:257])
                    if c % 2 == 0:
                        nc.scalar.activation(
                            C2Q[:, c, :],
                            cq[:, 0:D],
                            AF.Copy,
                            scale=linv[:, c : c + 1],
                        )
                    else:
                        nc.vector.tensor_scalar_mul(
                            C2Q[:, c, :], cq[:, 0:D], linv[:, c : c + 1]
                        )
                st_eng2 = nc.scalar if b >= bpc - 2 else nc.sync
                nc.sync.dma_start(Gb[:, 0:4, 1, :], C2Q[:, 0:4, :])
                st_eng2.dma_start(Gb[:, 4:8, 1, :], C2Q[:, 4:8, :])

                if PHASE < 9:
                    continue
                # ---- G3 = H * C2Q into G34, then store halves ----
                nc.vector.tensor_mul(
                    G34[:, 0:4, 0, :], Hb[:, 0:4, 0:D], C2Q[:, 0:4, :]
                )
                nc.gpsimd.tensor_mul(
                    G34[:, 4:8, 0, :], Hb[:, 4:8, 0:D], C2Q[:, 4:8, :]
                )
                nc.sync.dma_start(Gb[:, 0:4, 2:4, :], G34[:, 0:4])
                st_eng2.dma_start(Gb[:, 4:8, 2:4, :], G34[:, 4:8])

    return nc


_NC_CACHE = {}


def get_nc(bpc=BPC):
    key = (bpc, PHASE)
    if key not in _NC_CACHE:
        import bass_rust as _bass_rust

        nc = bass.Bass()
        build_kernel(nc, bpc)
        # TRN2 allows at most 1 sync wait per instruction (2 on event
        # semaphores); Tile emits more.  These are the bacc lowering passes
        # that legalize the wait lists.
        _bass_rust.move_matmul_waits_to_ldweights(nc.m)
        _bass_rust.generate_event_semaphores(nc)
        # lower bass_isa subclasses (e.g. EVENT_SEMAPHORE_RANGE_CLEAR) into
        # raw InstISA encodings walrus can emit
        mybir.codegen_inst_isa_subclasses(nc)
        _NC_CACHE[key] = nc
    return _NC_CACHE[key]


def _prep(inputs):
    import ml_dtypes

    bf16 = ml_dtypes.bfloat16
    H = np.asarray(inputs["H"], dtype=np.float32)
    U = np.asarray(inputs["U"], dtype=np.float32)
    w_h = np.asarray(inputs["w_h"], dtype=np.float32)
    w_u = np.asarray(inputs["w_u"], dtype=np.float32)
    w_hu = np.asarray(inputs["w_hu"], dtype=np.float32)

    # combined per-partition-contiguous H: [b, p, (c,d)] t-major aug | [kc, t] d-major
    Hbf = H.astype(bf16)
    Hf_ = np.ones((B, T, DA), dtype=bf16)
    Hf_[:, :, :D] = Hbf
    hb_part = Hf_.reshape(B, NT, P, DA).transpose(0, 2, 1, 3).reshape(B, P, NT * DA)
    ht_part = Hbf.transpose(0, 2, 1).reshape(B, 2, P, T).transpose(0, 2, 1, 3).reshape(B, P, 2 * T)
    HC = np.ascontiguousarray(np.concatenate([hb_part, ht_part], axis=2))
    # combined U: [b, p(j), d-aug] j-major | [p(d), kc, j] d-major
    Ubf = U.astype(bf16)
    Ub_ = np.ones((B, P, DA), dtype=bf16)
    Ub_[:, :, :D] = Ubf
    ut_part = Ubf.transpose(0, 2, 1).reshape(B, 2, P, P).transpose(0, 2, 1, 3).reshape(B, P, 2 * P)
    UC = np.ascontiguousarray(np.concatenate([Ub_, ut_part], axis=2))
    ones1 = np.ones((1, P), dtype=bf16)
    wu_col = np.ascontiguousarray(w_u.reshape(2, P).T[:, :, None]).astype(bf16)
    whu_wh = np.ascontiguousarray(
        np.stack([w_hu.reshape(2, P).T, w_h.reshape(2, P).T], axis=2)
    )
    return HC, UC, ones1, wu_col, whu_wh


def run(inputs, trace=False, **kwargs):
    from concourse.bass_utils import run_bass_kernel_spmd

    nc = get_nc(BPC)
    HC, UC, ones1, wu_col, whu_wh = _prep(inputs)
    in_maps = [
        {
            "HC": HC[c * BPC : (c + 1) * BPC],
            "UC": UC[c * BPC : (c + 1) * BPC],
            "ones1": ones1,
            "wu_col": wu_col,
            "whu_wh": whu_wh,
        }
        for c in range(NCORES)
    ]
    res = run_bass_kernel_spmd(
        nc, in_maps, core_ids=list(range(NCORES)), trace=trace, **kwargs
    )
    out = np.concatenate([res.results[c]["G"] for c in range(NCORES)], axis=0)
    return out, res


def kernel(**inputs):
    out, _ = run(inputs, trace=False)
    return out
